# revision 25
# baseline (speedup 1.0000x reference)
"""CooccurrenceEnhancer kernel for Trainium2 (8 NeuronCores, data-parallel).

Computes, for each token row b:
    y[b, :]  = sum_i scores[b, i] * cooc[ids[b, i], :]      (sparse @ dense)
    y[b, ids[b, :]] = -inf                                   (mask existing)
    top-32 (values, indices) of y[b, :]                      (sorted desc)
    output = concat(ids, top_idx), concat(scores, top_vals)

Strategy: batch is sharded across 8 cores (8192 tokens each, 64 tiles of
128).  Per tile: gpsimd.local_scatter builds the sparse score rows in two
fp16 planes (hi/lo split of the fp32 score; cooc is likewise split into
fp16 hi/lo of 256*cooc so all four planes are fp16).  PE transposes the
scatter output and accumulates three fp16 matmuls per K-chunk
(hi*chi + hi*clo + lo*chi) into fp32 PSUM, which reproduces the fp32
matmul to ~1 ulp.  The -big candidate mask is folded into the PSUM
accumulation as a 13th matmul (identity x mask plane) and ACT drains
PSUM->SBUF with the 1/256 descale folded into the copy, so the DVE runs
nothing but the eleven exact top-32 scans per tile (4x max8, 4x
find_index8, 3x match_replace8) -- the DVE is the bottleneck engine at
>92% occupancy and those scans are its ISA floor (1 elem/cycle, no fast
modes, 8 results per scan).  The index scans are deferred behind the
max8/match_replace chain to shorten the cross-round critical path, and
the first tile drains via DVE tensor_tensor (the ACT queue is busy with
transposecopies at startup).
"""

import numpy as np
from contextlib import ExitStack

from concourse import bacc, bass, mybir
from concourse import tile
from concourse import library_config
from concourse.bass_utils import run_bass_kernel_spmd

P = 128            # partitions / tokens per tile
E = 512            # number of experts
CAND = 32          # candidates per token
N_CORES = 8
B = 65536          # total tokens
TPC = B // N_CORES  # tokens per core
K_CHUNKS = E // P   # 4
TOPK = 32           # num_to_add = target_size(64) - CAND(32)
ROUNDS = TOPK // 8  # max8 yields 8 per round
MASK_VAL = -60000.0  # fp16-representable, dwarfs |y| <= ~16 after 256x scale
NEG_IMM = -1.0e30    # match_replace fill


def build_nc(ntiles: int = TPC // P):
    """Builds the single-core Bass program (same program runs on all cores)."""
    nc = bacc.Bacc("TRN2", target_bir_lowering=False, debug=False)
    f16 = mybir.dt.float16
    f32 = mybir.dt.float32

    tokens = ntiles * P
    ids_d = nc.dram_tensor("ids16", [tokens, CAND], mybir.dt.int16,
                           kind="ExternalInput").ap()
    shi_d = nc.dram_tensor("shi", [tokens, CAND], f16, kind="ExternalInput").ap()
    slo_d = nc.dram_tensor("slo", [tokens, CAND], f16, kind="ExternalInput").ap()
    chi_d = nc.dram_tensor("chi", [E, E], f16, kind="ExternalInput").ap()
    clo_d = nc.dram_tensor("clo", [E, E], f16, kind="ExternalInput").ap()
    ident_d = nc.dram_tensor("ident", [P, P], f16, kind="ExternalInput").ap()
    vals_d = nc.dram_tensor("out_vals", [tokens, TOPK], f32,
                            kind="ExternalOutput").ap()
    idx_d = nc.dram_tensor("out_idx", [tokens, TOPK], mybir.dt.uint16,
                           kind="ExternalOutput").ap()

    G = 4 if ntiles % 4 == 0 else 1  # tiles per DMA batch group
    ngroups = ntiles // G

    with tile.TileContext(nc) as tc, ExitStack() as ctx:
        const = ctx.enter_context(tc.tile_pool(name="const", bufs=1))
        inp = ctx.enter_context(tc.tile_pool(name="inp", bufs=4))
        scat = ctx.enter_context(tc.tile_pool(name="scat", bufs=6))
        stp = ctx.enter_context(tc.tile_pool(name="stp", bufs=4))
        ysb = ctx.enter_context(tc.tile_pool(name="ysb", bufs=4))
        outp = ctx.enter_context(tc.tile_pool(name="outp", bufs=6))
        psum = ctx.enter_context(tc.tile_pool(name="psum", bufs=4, space="PSUM"))
        pst = ctx.enter_context(tc.tile_pool(name="pst", bufs=4, space="PSUM"))

        nc.gpsimd.load_library(library_config.local_scatter)

        chi_sb = const.tile([P, K_CHUNKS, E], f16)
        clo_sb = const.tile([P, K_CHUNKS, E], f16)
        ident = const.tile([P, P], f16)
        negbig = const.tile([P, CAND], f16)
        def load_group(g):
            grows = slice(g * G * P, (g + 1) * G * P)
            ids_g = inp.tile([P, G, CAND], mybir.dt.int16, tag="ids",
                             name="ids_g")
            shi_g = inp.tile([P, G, CAND], f16, tag="shi", name="shi_g")
            slo_g = inp.tile([P, G, CAND], f16, tag="slo", name="slo_g")
            nc.sync.dma_start(
                out=ids_g[:], in_=ids_d[grows, :].rearrange("(f p) c -> p f c", p=P))
            nc.sync.dma_start(
                out=shi_g[:], in_=shi_d[grows, :].rearrange("(f p) c -> p f c", p=P))
            nc.sync.dma_start(
                out=slo_g[:], in_=slo_d[grows, :].rearrange("(f p) c -> p f c", p=P))
            return ids_g, shi_g, slo_g

        # Group-0 inputs are issued first on the sync queue; the constant
        # DMAs are spread over the scalar and sync queues afterwards
        # (gpsimd must stay free for the group-0 scatters; the constants
        # arrive while the scatters run).
        g0_tiles = load_group(0)
        nc.scalar.dma_start(out=ident[:], in_=ident_d[:])
        for k in range(K_CHUNKS):
            eng = nc.scalar if k < 2 else nc.sync
            eng.dma_start(out=chi_sb[:, k, :], in_=chi_d[k * P:(k + 1) * P, :])
            eng.dma_start(out=clo_sb[:, k, :], in_=clo_d[k * P:(k + 1) * P, :])
        nc.vector.memset(negbig[:], MASK_VAL)

        for g in range(ngroups):
            grows = slice(g * G * P, (g + 1) * G * P)
            ids_g, shi_g, slo_g = g0_tiles if g == 0 else load_group(g)

            for j in range(G):
                first_tile = (g == 0 and j == 0)
                vals_t = outp.tile([P, TOPK], f32, tag="vals")
                idx_t = outp.tile([P, TOPK], mybir.dt.uint16, tag="idx")
                ids_t = ids_g[:, j, :]
                s_hi = scat.tile([P, E], f16, tag="s_hi")
                s_lo = scat.tile([P, E], f16, tag="s_lo")
                mask = scat.tile([P, E], f16, tag="mask")
                nc.gpsimd.local_scatter(s_hi[:], shi_g[:, j, :], ids_t,
                                        channels=P, num_elems=E, num_idxs=CAND)
                nc.gpsimd.local_scatter(s_lo[:], slo_g[:, j, :], ids_t,
                                        channels=P, num_elems=E, num_idxs=CAND)
                nc.gpsimd.local_scatter(mask[:], negbig[:], ids_t,
                                        channels=P, num_elems=E, num_idxs=CAND)

                # Transpose the two scatter planes chunk-by-chunk (PE).
                # All 8 transposes pack into one PSUM bank; one wide ACT
                # copy drains them to SBUF (hi chunks even, lo chunks odd).
                st = stp.tile([P, 2 * K_CHUNKS, P], f16, tag="st")
                pt = pst.tile([P, 2 * K_CHUNKS, P], f16, tag="pt")
                for k in range(K_CHUNKS):
                    nc.tensor.transpose(pt[:, 2 * k, :],
                                        s_hi[:, k * P:(k + 1) * P], ident[:])
                    nc.tensor.transpose(pt[:, 2 * k + 1, :],
                                        s_lo[:, k * P:(k + 1) * P], ident[:])
                nc.scalar.copy(st[:], pt[:])

                # y = S_hi @ chi + S_hi @ clo + S_lo @ chi  (fp32 PSUM accum)
                y_ps = psum.tile([P, E], f32, tag="y")
                mm = 0
                for k in range(K_CHUNKS):
                    for lhsT, rhs in ((st[:, 2 * k, :], chi_sb),
                                      (st[:, 2 * k, :], clo_sb),
                                      (st[:, 2 * k + 1, :], chi_sb)):
                        nc.tensor.matmul(y_ps[:], lhsT, rhs[:, k, :],
                                         start=(mm == 0),
                                         stop=(mm == 11 and first_tile))
                        mm += 1

                bufs = [ysb.tile([P, E], f32, tag=f"y{r}", name=f"y{r}")
                        for r in range(ROUNDS)]
                if first_tile:
                    # Latency special case: the very first tile's drain runs
                    # on the (still idle) DVE instead of queueing behind the
                    # ACT transposecopies; it scans in the scaled domain and
                    # descales its 32 outputs at the end.
                    nc.vector.tensor_tensor(out=bufs[0][:], in0=y_ps[:],
                                            in1=mask[:],
                                            op=mybir.AluOpType.add)
                else:
                    # fold the candidate mask into the PSUM accumulation (PE
                    # adds the scattered mask plane through the identity),
                    # then ACT drains PSUM->SBUF with the 1/256 descale
                    # folded in.  DVE is left with only the
                    # max8/find_index8/match_replace8 scans.
                    nc.tensor.matmul(y_ps[:], ident[:], mask[:], start=False,
                                     stop=True)
                    nc.scalar.mul(bufs[0][:], y_ps[:], 1.0 / 256.0)

                # The per-round exclusion is a one-op threshold-select on
                # DVE: y' = (y < tau) * y with tau = the round's 8th value.
                # It lowers to TensorScalarPtr, which (unlike match_replace)
                # runs in the 2x_2p fast mode for fp32 SBUF operands -- half
                # the cycles and no match-value load.  Survivors stay
                # positive (y > 0 strictly), excluded elements become 0,
                # masked candidates stay at -big.
                for r in range(ROUNDS):
                    v_sl = vals_t[:, r * 8:(r + 1) * 8]
                    nc.vector.max(v_sl, bufs[r][:])
                    if r < ROUNDS - 1:
                        nc.vector.scalar_tensor_tensor(
                            out=bufs[r + 1][:], in0=bufs[r][:],
                            scalar=v_sl[:, 7:8], in1=bufs[r][:],
                            op0=mybir.AluOpType.is_lt,
                            op1=mybir.AluOpType.mult)
                    nc.vector.max_index(idx_t[:, r * 8:(r + 1) * 8],
                                        v_sl, bufs[r][:])
                if first_tile:
                    nc.vector.tensor_scalar_mul(vals_t[:], vals_t[:],
                                                1.0 / 256.0)

                trows = slice(g * G * P + j * P, g * G * P + (j + 1) * P)
                nc.sync.dma_start(out=vals_d[trows, :], in_=vals_t[:])
                nc.sync.dma_start(out=idx_d[trows, :], in_=idx_t[:])

    nc.compile()
    return nc


def host_prep(candidate_ids, candidate_scores, cooccurrence):
    """Dedup ids per row (summing duplicate scores), fp16-split scores and
    256*cooc.  Returns per-core input maps (plus shared constants)."""
    ids = np.asarray(candidate_ids).astype(np.int32)
    s = np.asarray(candidate_scores).astype(np.float32)
    C = np.asarray(cooccurrence).astype(np.float32)
    nb, cand = ids.shape

    order = np.argsort(ids, axis=1, kind="stable")
    ids_s = np.take_along_axis(ids, order, axis=1)
    s_s = np.take_along_axis(s, order, axis=1)
    first = np.ones_like(ids_s, dtype=bool)
    first[:, 1:] = ids_s[:, 1:] != ids_s[:, :-1]
    grp = np.cumsum(first, axis=1) - 1
    rows = np.repeat(np.arange(nb), cand)
    sums = np.zeros((nb, cand), np.float32)
    np.add.at(sums, (rows, grp.ravel()), s_s.ravel())
    dids = np.full((nb, cand), -1, np.int16)
    rr, cc = np.nonzero(first)
    dids[rr, grp[rr, cc]] = ids_s[rr, cc].astype(np.int16)
    valid = dids >= 0
    sums = np.where(valid, sums, 0).astype(np.float32)

    shi = sums.astype(np.float16)
    slo = (sums - shi.astype(np.float32)).astype(np.float16)
    Cs = (C * np.float32(256.0)).astype(np.float32)
    chi = Cs.astype(np.float16)
    clo = (Cs - chi.astype(np.float32)).astype(np.float16)
    ident = np.eye(P, dtype=np.float16)

    in_maps = []
    for c in range(N_CORES):
        sh = slice(c * TPC, (c + 1) * TPC)
        in_maps.append({
            "ids16": np.ascontiguousarray(dids[sh]),
            "shi": np.ascontiguousarray(shi[sh]),
            "slo": np.ascontiguousarray(slo[sh]),
            "chi": chi,
            "clo": clo,
            "ident": ident,
        })
    return in_maps


_NC_CACHE = {}


def _get_nc(ntiles):
    if ntiles not in _NC_CACHE:
        _NC_CACHE[ntiles] = build_nc(ntiles)
    return _NC_CACHE[ntiles]


def run_device(in_maps, trace=False, ntiles=TPC // P):
    nc = _get_nc(ntiles)
    return run_bass_kernel_spmd(nc, in_maps, list(range(len(in_maps))),
                                trace=trace)


def kernel(candidate_ids, candidate_scores, cooccurrence, target_size):
    ids = np.asarray(candidate_ids)
    s = np.asarray(candidate_scores).astype(np.float32)
    in_maps = host_prep(ids, s, cooccurrence)
    br = run_device(in_maps)
    vals = np.concatenate([br.results[c]["out_vals"] for c in range(N_CORES)], 0)
    idx = np.concatenate([br.results[c]["out_idx"] for c in range(N_CORES)], 0)
    add_ids = idx.astype(ids.dtype)
    expanded_ids = np.concatenate([ids, add_ids], axis=1)
    expanded_scores = np.concatenate([s, vals], axis=1)
    return expanded_ids, expanded_scores



# revision 26
# speedup vs baseline: 1.1956x; 1.1956x over previous
"""CooccurrenceEnhancer kernel for Trainium2 (8 NeuronCores, data-parallel).

Computes, for each token row b:
    y[b, :]  = sum_i scores[b, i] * cooc[ids[b, i], :]      (sparse @ dense)
    y[b, ids[b, :]] = -inf                                   (mask existing)
    top-32 (values, indices) of y[b, :]                      (sorted desc)
    output = concat(ids, top_idx), concat(scores, top_vals)

Strategy: batch is sharded across 8 cores (8192 tokens each, 64 tiles of
128).  Per tile: gpsimd.local_scatter builds the sparse score rows in two
fp16 planes (hi/lo split of the fp32 score; cooc is likewise split into
fp16 hi/lo of 256*cooc so all four planes are fp16).  PE transposes the
scatter output and accumulates three fp16 matmuls per K-chunk
(hi*chi + hi*clo + lo*chi) into fp32 PSUM, which reproduces the fp32
matmul to ~1 ulp.  The -big candidate mask is folded into the PSUM
accumulation as a 13th matmul (identity x mask plane) and ACT drains
PSUM->SBUF with the 1/256 descale folded into the copy, so the DVE runs
nothing but the eleven exact top-32 scans per tile (4x max8, 4x
find_index8, 3x match_replace8) -- the DVE is the bottleneck engine at
>92% occupancy and those scans are its ISA floor (1 elem/cycle, no fast
modes, 8 results per scan).  The index scans are deferred behind the
max8/match_replace chain to shorten the cross-round critical path, and
the first tile drains via DVE tensor_tensor (the ACT queue is busy with
transposecopies at startup).
"""

import numpy as np
from contextlib import ExitStack

from concourse import bacc, bass, mybir
from concourse import tile
from concourse import library_config
from concourse.bass_utils import run_bass_kernel_spmd

P = 128            # partitions / tokens per tile
E = 512            # number of experts
CAND = 32          # candidates per token
N_CORES = 8
B = 65536          # total tokens
TPC = B // N_CORES  # tokens per core
K_CHUNKS = E // P   # 4
TOPK = 32           # num_to_add = target_size(64) - CAND(32)
ROUNDS = TOPK // 8  # max8 yields 8 per round
MASK_VAL = -60000.0  # fp16-representable, dwarfs |y| <= ~16 after 256x scale
NEG_IMM = -1.0e30    # match_replace fill


def build_nc(ntiles: int = TPC // P):
    """Builds the single-core Bass program (same program runs on all cores)."""
    nc = bacc.Bacc("TRN2", target_bir_lowering=False, debug=False)
    f16 = mybir.dt.float16
    f32 = mybir.dt.float32

    tokens = ntiles * P
    ids_d = nc.dram_tensor("ids16", [tokens, CAND], mybir.dt.int16,
                           kind="ExternalInput").ap()
    shi_d = nc.dram_tensor("shi", [tokens, CAND], f16, kind="ExternalInput").ap()
    slo_d = nc.dram_tensor("slo", [tokens, CAND], f16, kind="ExternalInput").ap()
    chi_d = nc.dram_tensor("chi", [E, E], f16, kind="ExternalInput").ap()
    clo_d = nc.dram_tensor("clo", [E, E], f16, kind="ExternalInput").ap()
    ident_d = nc.dram_tensor("ident", [P, P], f16, kind="ExternalInput").ap()
    vals_d = nc.dram_tensor("out_vals", [tokens, TOPK], f32,
                            kind="ExternalOutput").ap()
    idx_d = nc.dram_tensor("out_idx", [tokens, TOPK], mybir.dt.uint16,
                           kind="ExternalOutput").ap()

    G = 4 if ntiles % 4 == 0 else 1  # tiles per DMA batch group
    ngroups = ntiles // G

    with tile.TileContext(nc) as tc, ExitStack() as ctx:
        const = ctx.enter_context(tc.tile_pool(name="const", bufs=1))
        inp = ctx.enter_context(tc.tile_pool(name="inp", bufs=4))
        scat = ctx.enter_context(tc.tile_pool(name="scat", bufs=6))
        stp = ctx.enter_context(tc.tile_pool(name="stp", bufs=4))
        ysb = ctx.enter_context(tc.tile_pool(name="ysb", bufs=4))
        outp = ctx.enter_context(tc.tile_pool(name="outp", bufs=6))
        psum = ctx.enter_context(tc.tile_pool(name="psum", bufs=4, space="PSUM"))
        pst = ctx.enter_context(tc.tile_pool(name="pst", bufs=4, space="PSUM"))

        nc.gpsimd.load_library(library_config.local_scatter)

        chi_sb = const.tile([P, K_CHUNKS, E], f16)
        clo_sb = const.tile([P, K_CHUNKS, E], f16)
        ident = const.tile([P, P], f16)
        negbig = const.tile([P, CAND], f16)
        def load_group(g):
            grows = slice(g * G * P, (g + 1) * G * P)
            ids_g = inp.tile([P, G, CAND], mybir.dt.int16, tag="ids",
                             name="ids_g")
            shi_g = inp.tile([P, G, CAND], f16, tag="shi", name="shi_g")
            slo_g = inp.tile([P, G, CAND], f16, tag="slo", name="slo_g")
            nc.sync.dma_start(
                out=ids_g[:], in_=ids_d[grows, :].rearrange("(f p) c -> p f c", p=P))
            nc.sync.dma_start(
                out=shi_g[:], in_=shi_d[grows, :].rearrange("(f p) c -> p f c", p=P))
            nc.sync.dma_start(
                out=slo_g[:], in_=slo_d[grows, :].rearrange("(f p) c -> p f c", p=P))
            return ids_g, shi_g, slo_g

        # Group-0 inputs are issued first on the sync queue; the constant
        # DMAs are spread over the scalar and sync queues afterwards
        # (gpsimd must stay free for the group-0 scatters; the constants
        # arrive while the scatters run).
        g0_tiles = load_group(0)
        nc.scalar.dma_start(out=ident[:], in_=ident_d[:])
        for k in range(K_CHUNKS):
            eng = nc.scalar if k < 2 else nc.sync
            eng.dma_start(out=chi_sb[:, k, :], in_=chi_d[k * P:(k + 1) * P, :])
            eng.dma_start(out=clo_sb[:, k, :], in_=clo_d[k * P:(k + 1) * P, :])
        nc.vector.memset(negbig[:], MASK_VAL)

        for g in range(ngroups):
            grows = slice(g * G * P, (g + 1) * G * P)
            ids_g, shi_g, slo_g = g0_tiles if g == 0 else load_group(g)

            for j in range(G):
                first_tile = (g == 0 and j == 0)
                vals_t = outp.tile([P, TOPK], f32, tag="vals")
                idx_t = outp.tile([P, TOPK], mybir.dt.uint16, tag="idx")
                ids_t = ids_g[:, j, :]
                s_hi = scat.tile([P, E], f16, tag="s_hi")
                s_lo = scat.tile([P, E], f16, tag="s_lo")
                mask = scat.tile([P, E], f16, tag="mask")
                nc.gpsimd.local_scatter(s_hi[:], shi_g[:, j, :], ids_t,
                                        channels=P, num_elems=E, num_idxs=CAND)
                nc.gpsimd.local_scatter(s_lo[:], slo_g[:, j, :], ids_t,
                                        channels=P, num_elems=E, num_idxs=CAND)
                nc.gpsimd.local_scatter(mask[:], negbig[:], ids_t,
                                        channels=P, num_elems=E, num_idxs=CAND)

                # Transpose the two scatter planes chunk-by-chunk (PE).
                # All 8 transposes pack into one PSUM bank; one wide ACT
                # copy drains them to SBUF (hi chunks even, lo chunks odd).
                st = stp.tile([P, 2 * K_CHUNKS, P], f16, tag="st")
                pt = pst.tile([P, 2 * K_CHUNKS, P], f16, tag="pt")
                for k in range(K_CHUNKS):
                    nc.tensor.transpose(pt[:, 2 * k, :],
                                        s_hi[:, k * P:(k + 1) * P], ident[:])
                    nc.tensor.transpose(pt[:, 2 * k + 1, :],
                                        s_lo[:, k * P:(k + 1) * P], ident[:])
                nc.scalar.copy(st[:], pt[:])

                # y = S_hi @ chi + S_hi @ clo + S_lo @ chi  (fp32 PSUM accum)
                y_ps = psum.tile([P, E], f32, tag="y")
                mm = 0
                for k in range(K_CHUNKS):
                    for lhsT, rhs in ((st[:, 2 * k, :], chi_sb),
                                      (st[:, 2 * k, :], clo_sb),
                                      (st[:, 2 * k + 1, :], chi_sb)):
                        nc.tensor.matmul(y_ps[:], lhsT, rhs[:, k, :],
                                         start=(mm == 0),
                                         stop=(mm == 11 and first_tile))
                        mm += 1

                bufs = [ysb.tile([P, E], f32, tag=f"y{r}", name=f"y{r}")
                        for r in range(ROUNDS)]
                if first_tile:
                    # Latency special case: the very first tile's drain runs
                    # on the (still idle) DVE instead of queueing behind the
                    # ACT transposecopies; it scans in the scaled domain and
                    # descales its 32 outputs at the end.
                    nc.vector.tensor_tensor(out=bufs[0][:], in0=y_ps[:],
                                            in1=mask[:],
                                            op=mybir.AluOpType.add)
                else:
                    # fold the candidate mask into the PSUM accumulation (PE
                    # adds the scattered mask plane through the identity),
                    # then ACT drains PSUM->SBUF with the 1/256 descale
                    # folded in.  DVE is left with only the
                    # max8/find_index8/match_replace8 scans.
                    nc.tensor.matmul(y_ps[:], ident[:], mask[:], start=False,
                                     stop=True)
                    nc.scalar.mul(bufs[0][:], y_ps[:], 1.0 / 256.0)

                # max8/match_replace first (the critical chain to the next
                # round); the index searches are deferred so they fill DVE
                # slots off the critical path -- except on the last tile,
                # where deferring would lengthen the kernel tail.
                # (Measured dead ends: threshold-exclusion via
                # scalar_tensor_tensor runs at 1 elem/cycle on HW -- the
                # 2x_2p mode does not engage for the two-stream form -- and
                # gpsimd rejects the AP-scalar variant entirely, so
                # match_replace at 1 elem/cycle is optimal here.)
                defer = not (g == ngroups - 1 and j == G - 1)
                for r in range(ROUNDS):
                    v_sl = vals_t[:, r * 8:(r + 1) * 8]
                    nc.vector.max(v_sl, bufs[r][:])
                    if not defer:
                        nc.vector.max_index(idx_t[:, r * 8:(r + 1) * 8],
                                            v_sl, bufs[r][:])
                    if r < ROUNDS - 1:
                        nc.vector.match_replace(bufs[r + 1][:], v_sl,
                                                bufs[r][:], NEG_IMM)
                if defer:
                    for r in range(ROUNDS):
                        nc.vector.max_index(idx_t[:, r * 8:(r + 1) * 8],
                                            vals_t[:, r * 8:(r + 1) * 8],
                                            bufs[r][:])
                if first_tile:
                    nc.vector.tensor_scalar_mul(vals_t[:], vals_t[:],
                                                1.0 / 256.0)

                trows = slice(g * G * P + j * P, g * G * P + (j + 1) * P)
                nc.sync.dma_start(out=vals_d[trows, :], in_=vals_t[:])
                nc.sync.dma_start(out=idx_d[trows, :], in_=idx_t[:])

    nc.compile()
    return nc


def host_prep(candidate_ids, candidate_scores, cooccurrence):
    """Dedup ids per row (summing duplicate scores), fp16-split scores and
    256*cooc.  Returns per-core input maps (plus shared constants)."""
    ids = np.asarray(candidate_ids).astype(np.int32)
    s = np.asarray(candidate_scores).astype(np.float32)
    C = np.asarray(cooccurrence).astype(np.float32)
    nb, cand = ids.shape

    order = np.argsort(ids, axis=1, kind="stable")
    ids_s = np.take_along_axis(ids, order, axis=1)
    s_s = np.take_along_axis(s, order, axis=1)
    first = np.ones_like(ids_s, dtype=bool)
    first[:, 1:] = ids_s[:, 1:] != ids_s[:, :-1]
    grp = np.cumsum(first, axis=1) - 1
    rows = np.repeat(np.arange(nb), cand)
    sums = np.zeros((nb, cand), np.float32)
    np.add.at(sums, (rows, grp.ravel()), s_s.ravel())
    dids = np.full((nb, cand), -1, np.int16)
    rr, cc = np.nonzero(first)
    dids[rr, grp[rr, cc]] = ids_s[rr, cc].astype(np.int16)
    valid = dids >= 0
    sums = np.where(valid, sums, 0).astype(np.float32)

    shi = sums.astype(np.float16)
    slo = (sums - shi.astype(np.float32)).astype(np.float16)
    Cs = (C * np.float32(256.0)).astype(np.float32)
    chi = Cs.astype(np.float16)
    clo = (Cs - chi.astype(np.float32)).astype(np.float16)
    ident = np.eye(P, dtype=np.float16)

    in_maps = []
    for c in range(N_CORES):
        sh = slice(c * TPC, (c + 1) * TPC)
        in_maps.append({
            "ids16": np.ascontiguousarray(dids[sh]),
            "shi": np.ascontiguousarray(shi[sh]),
            "slo": np.ascontiguousarray(slo[sh]),
            "chi": chi,
            "clo": clo,
            "ident": ident,
        })
    return in_maps


_NC_CACHE = {}


def _get_nc(ntiles):
    if ntiles not in _NC_CACHE:
        _NC_CACHE[ntiles] = build_nc(ntiles)
    return _NC_CACHE[ntiles]


def run_device(in_maps, trace=False, ntiles=TPC // P):
    nc = _get_nc(ntiles)
    return run_bass_kernel_spmd(nc, in_maps, list(range(len(in_maps))),
                                trace=trace)


def kernel(candidate_ids, candidate_scores, cooccurrence, target_size):
    ids = np.asarray(candidate_ids)
    s = np.asarray(candidate_scores).astype(np.float32)
    in_maps = host_prep(ids, s, cooccurrence)
    br = run_device(in_maps)
    vals = np.concatenate([br.results[c]["out_vals"] for c in range(N_CORES)], 0)
    idx = np.concatenate([br.results[c]["out_idx"] for c in range(N_CORES)], 0)
    add_ids = idx.astype(ids.dtype)
    expanded_ids = np.concatenate([ids, add_ids], axis=1)
    expanded_scores = np.concatenate([s, vals], axis=1)
    return expanded_ids, expanded_scores

